# revision 20
# baseline (speedup 1.0000x reference)
"""Trainium2 Bass kernel for CausalAttentionSortNet (bucketed causal sort-net scores).

Math (per bh slice; n=8192, bucket=64, nb=128 buckets, d=64):
  sq[i]  = cumavg(q)[64*i]            = (sum_{s<=64i} q[s]) / (64i+1)
  sk[j]  = sum_sigma cumavg(k)[64j+s] = H_j * Bk[j] + sum_s G[j,s] k[64j+s]
           where Bk[j] = sum of full buckets < j, H_j = sum_s 1/(64j+s+1),
           G[j,s] = sum_{s'>=s} 1/(64j+s'+1)
  R[i,jj] = (sq[i] . skp[jj]) / 8 ; skp = [0, sk[0..126]] padded front
  masked softmax over jj<=i, then keep strictly jj<i.

Layout trick: per bh, DMA q/k as [128 partitions, 4096] where partition
p = 4*jj + c holds seq rows 64*(32t+jj) + 16c + s (t in free dim) -> each
partition reads 4KB-contiguous HBM chunks. Bucket sums are then a PE
matmul with a 0/1 quarter-fold stationary (collapsing the 4 quarters c)
followed by a short DVE reduce over s (16 strided elems). Prefix-over-
buckets, transposes and the final 128x129 score matmul all run on PE.
"""

import numpy as np
from contextlib import ExitStack

import concourse.bass as bass
import concourse.tile as tile
from concourse.tile import add_dep_helper
from concourse import mybir
from concourse import bass_utils

# ---------------- problem constants (hardcoded per spec) ----------------
BH_TOTAL = 32
N_CORES = 8
BH = BH_TOTAL // N_CORES          # 4 bh slices per core
SEQ = 8192
D = 64
BUCKET = 64
NB = SEQ // BUCKET                # 128 buckets
NJ = NB + 1                       # 129 output cols
NEG = -1e30

_F32 = mybir.dt.float32


def _host_constants():
    inv = 1.0 / np.arange(1, SEQ + 1, dtype=np.float64)          # 1/(t+1)
    invb = inv.reshape(NB, BUCKET)                               # [j, s]
    H = invb.sum(axis=1)                                         # [128]
    # suffix sums within bucket: G[j, s] = sum_{s'>=s} inv[j, s']
    G = np.cumsum(invb[:, ::-1], axis=1)[:, ::-1]                # [128, 64]

    i_idx = np.arange(NB)
    c8 = 1.0 / (8.0 * (BUCKET * i_idx + 1))                      # c_i/8
    j_col = i_idx[:, None]
    i_row = i_idx[None, :]
    pmq = np.where(j_col < i_row, c8[None, :], 0.0)              # [j, i]
    pmk = np.where(j_col < i_row, H[None, :], 0.0)               # [j, j2]

    # gw[p, 1024*t + 64*s + d] = G[32t + p//4, 16*(p%4) + s]
    p = np.arange(128)
    t = np.arange(4)
    s = np.arange(16)
    jj = 32 * t[None, :, None] + (p // 4)[:, None, None]         # [p, t, 1]
    sg = (16 * (p % 4))[:, None, None] + s[None, None, :]        # [p, 1, s]
    gw = G[jj, sg]                                               # [128, 4, 16]
    gw = np.repeat(gw.reshape(128, 64)[:, :, None], D, axis=2).reshape(128, 4096)

    qsel = (p[:, None] // 4 == np.arange(32)[None, :]).astype(np.float64)

    ident = np.eye(128)

    jj_col = np.arange(NJ)[None, :]
    i_rows = np.arange(NB)[:, None]
    maskneg = np.where(jj_col <= i_rows, 0.0, NEG)               # [128, 129]
    maskstrict = (jj_col < i_rows).astype(np.float64)            # [128, 129]

    f = np.float32
    return dict(
        gw=gw.astype(f), pmq=pmq.astype(f), pmk=pmk.astype(f),
        cq8=c8.astype(f).reshape(128, 1), qsel=qsel.astype(f),
        ident=ident.astype(f), maskneg=maskneg.astype(f),
        maskstrict=maskstrict.astype(f),
    )


def _build_program():
    nc = bass.Bass("TRN2", target_bir_lowering=False, debug=False)

    q_t = nc.dram_tensor("q", [BH, SEQ, D], _F32, kind="ExternalInput")
    k_t = nc.dram_tensor("k", [BH, SEQ, D], _F32, kind="ExternalInput")
    gw_t = nc.dram_tensor("gw", [128, 4096], _F32, kind="ExternalInput")
    pmq_t = nc.dram_tensor("pmq", [128, 128], _F32, kind="ExternalInput")
    pmk_t = nc.dram_tensor("pmk", [128, 128], _F32, kind="ExternalInput")
    cq8_t = nc.dram_tensor("cq8", [128, 1], _F32, kind="ExternalInput")
    qsel_t = nc.dram_tensor("qsel", [128, 32], _F32, kind="ExternalInput")
    id_t = nc.dram_tensor("ident", [128, 128], _F32, kind="ExternalInput")
    mn_t = nc.dram_tensor("maskneg", [128, NJ], _F32, kind="ExternalInput")
    ms_t = nc.dram_tensor("maskstrict", [128, NJ], _F32, kind="ExternalInput")
    out_t = nc.dram_tensor("out", [BH, NB, NJ], _F32, kind="ExternalOutput")

    with tile.TileContext(nc) as tc, ExitStack() as ctx:
        _body(ctx, tc,
              q_t.ap(), k_t.ap(), out_t.ap(),
              gw_t.ap(), pmq_t.ap(), pmk_t.ap(), cq8_t.ap(), qsel_t.ap(),
              id_t.ap(), mn_t.ap(), ms_t.ap())
    _split_matmul_waits(nc)
    return nc


_NO_SPLIT = ()


def _split_matmul_waits(nc):
    """This walrus build rejects compute instructions carrying more than one
    sync wait. Moving the waits onto single-wait NoOps placed immediately
    before the instruction in the same engine queue is semantically
    identical: the sequencer executes waits in queue order before
    dispatching."""
    n = 0
    for f in nc.m.functions:
        for b in f.blocks:
            insts = list(b.instructions)
            out = []
            changed = False
            for i in insts:
                si = getattr(i, "sync_info", None)
                if (si is not None and len(si.on_wait) > 1
                        and type(i).__name__ not in _NO_SPLIT
                        and i.is_executable()):
                    n += 1
                    changed = True
                    for wi, w in enumerate(si.on_wait):
                        nop = mybir.InstNoOp(
                            name=f"{i.name}-wsplit{wi}", ins=[], outs=[])
                        nop.engine = i.engine
                        nop.sync_info = mybir.SyncInfo(on_wait=[w], on_update=[])
                        out.append(nop)
                    i.sync_info = mybir.SyncInfo(
                        on_wait=[], on_update=list(si.on_update))
                out.append(i)
            if changed:
                b.instructions = out
    return n


def _body(ctx, tc, q, k, out, gw_d, pmq_d, pmk_d, cq8_d, qsel_d, id_d, mn_d, ms_d):
    nc = tc.nc
    cpool = ctx.enter_context(tc.tile_pool(name="consts", bufs=1))
    dpool = ctx.enter_context(tc.tile_pool(name="data", bufs=2))
    spool = ctx.enter_context(tc.tile_pool(name="small", bufs=2))
    ppool = ctx.enter_context(tc.tile_pool(name="psum", bufs=1, space="PSUM"))

    # ---- resident constants ----
    const_dmas = []

    def _const(shape, src, tag):
        t = cpool.tile(shape, _F32, tag=tag)
        const_dmas.append(nc.sync.dma_start(t[:], src))
        return t

    gw = _const([128, 4096], gw_d, "gw")
    pmq = _const([128, 128], pmq_d, "pmq")
    pmk = _const([128, 128], pmk_d, "pmk")
    cq8 = _const([128, 1], cq8_d, "cq8")
    qsel = _const([128, 32], qsel_d, "qsel")
    ident = _const([128, 128], id_d, "ident")
    maskneg = _const([128, NJ], mn_d, "maskneg")
    maskstrict = _const([128, NJ], ms_d, "maskstrict")



    for bh in range(BH):
        # seq rows: 2048t + 64jj + 16c + s ; partition = 4jj+c ; free = (t,s,d)
        def _src(x):
            v = x[bh].rearrange("(t jj c s) d -> t jj c s d", t=4, jj=32, c=4, s=16)
            return v.transpose([1, 2, 0, 3, 4]).rearrange("jj c t s d -> (jj c) t s d")

        def _dst(tl):
            return tl[:].rearrange("p (t s d) -> p t s d", t=4, s=16, d=64)

        qt = dpool.tile([128, 4096], _F32, tag="qt")
        dma_qt = nc.sync.dma_start(_dst(qt), _src(q))
        kt = dpool.tile([128, 4096], _F32, tag="kt")
        dma_kt = nc.sync.dma_start(_dst(kt), _src(k))
        qf = spool.tile([128, D], _F32, tag="qf")   # q[64j, :] per bucket j
        nc.sync.dma_start(qf[:], q[bh].rearrange("(j r) d -> j r d", r=64)[:, 0, :])

        kw = dpool.tile([128, 4096], _F32, tag="kw")
        nc.gpsimd.tensor_mul(kw[:], kt[:], gw[:])

        # ---- phase 1: quarter-fold on PE ----
        psq = ppool.tile([128, 1024], _F32, tag="psq")
        psk = ppool.tile([128, 1024], _F32, tag="psk")
        pskw = ppool.tile([128, 1024], _F32, tag="pskw")
        for tau in range(4):
            for h in range(2):
                src = slice(1024 * tau + 512 * h, 1024 * tau + 512 * (h + 1))
                dst = slice(512 * h, 512 * (h + 1))
                rows = slice(32 * tau, 32 * tau + 32)
                tp = (0, 32 * tau)
                for ps, data in ((psq, qt), (psk, kt), (pskw, kw)):
                    nc.tensor.matmul(ps[rows, dst], qsel[:], data[:, src],
                                     start=True, stop=True, tile_position=tp)

        # ---- phase 2: finish bucket sums over s (16 strided) on DVE ----
        qb = spool.tile([128, D], _F32, tag="qb")
        nc.vector.reduce_sum(qb[:], psq[:].rearrange("p (s d) -> p d s", s=16, d=64),
                             axis=mybir.AxisListType.X)
        kb = spool.tile([128, D], _F32, tag="kb")
        nc.vector.reduce_sum(kb[:], psk[:].rearrange("p (s d) -> p d s", s=16, d=64),
                             axis=mybir.AxisListType.X)
        kg = spool.tile([128, D], _F32, tag="kg")
        nc.vector.reduce_sum(kg[:], pskw[:].rearrange("p (s d) -> p d s", s=16, d=64),
                             axis=mybir.AxisListType.X)

        # ---- phase 3: prefix over buckets (PE), then combine ----
        paccs = ppool.tile([128, 128], _F32, tag="paccs")
        nc.tensor.matmul(paccs[:, 0:64], pmq[:], qb[:], start=True, stop=True)
        nc.tensor.matmul(paccs[:, 64:128], pmk[:], kb[:], start=True, stop=True)

        sq = spool.tile([128, D], _F32, tag="sq")
        # sq = qf * (c/8) + Sq_scaled
        nc.vector.scalar_tensor_tensor(sq[:], qf[:], cq8[:], paccs[:, 0:64],
                                       op0=mybir.AluOpType.mult,
                                       op1=mybir.AluOpType.add)
        sk = spool.tile([128, D], _F32, tag="sk")
        nc.vector.tensor_add(sk[:], paccs[:, 64:128], kg[:])

        # ---- phase 4: transposes + score matmul ----
        ptr = ppool.tile([128, 512], _F32, tag="ptr")
        nc.tensor.transpose(ptr[0:64, 256:384], sq[:], ident[:])
        sqT = spool.tile([64, 128], _F32, tag="sqT")
        nc.vector.tensor_copy(sqT[:], ptr[0:64, 256:384])
        nc.tensor.transpose(ptr[0:64, 384:512], sk[:], ident[:])
        skpT = spool.tile([64, NJ + 3], _F32, tag="skpT")
        nc.vector.memset(skpT[:, 0:1], 0.0)
        nc.vector.tensor_copy(skpT[:, 1:129], ptr[0:64, 384:512])

        nc.tensor.matmul(ptr[:, 0:NJ], sqT[:], skpT[:, 0:NJ], start=True, stop=True)

        # ---- phase 5: masked softmax ----
        Rm = spool.tile([128, NJ], _F32, tag="Rm")
        nc.vector.tensor_add(Rm[:], ptr[:, 0:NJ], maskneg[:])
        nm = spool.tile([128, 1], _F32, tag="nm")
        nc.vector.reduce_max(nm[:], Rm[:], axis=mybir.AxisListType.X, negate=True)
        e = spool.tile([128, NJ], _F32, tag="e")
        den = spool.tile([128, 1], _F32, tag="den")
        nc.scalar.activation(e[:], Rm[:], mybir.ActivationFunctionType.Exp,
                             bias=nm[:], scale=1.0, accum_out=den[:])
        rden = spool.tile([128, 1], _F32, tag="rden")
        nc.vector.reciprocal(rden[:], den[:])
        outb = spool.tile([128, NJ], _F32, tag="outb")
        nc.vector.scalar_tensor_tensor(outb[:], e[:], rden[:], maskstrict[:],
                                       op0=mybir.AluOpType.mult,
                                       op1=mybir.AluOpType.mult)
        nc.sync.dma_start(out[bh], outb[:])


_CACHE = {}


def _get_program():
    if "nc" not in _CACHE:
        _CACHE["nc"] = _build_program()
        _CACHE["consts"] = _host_constants()
    return _CACHE["nc"], _CACHE["consts"]


def _get_runner():
    """Build the sharded PJRT callable once and cache it (mirrors
    bass2jax.run_bass_via_pjrt but reuses the jitted function across
    calls)."""
    if "runner" in _CACHE:
        return _CACHE["runner"]
    import jax
    from jax.sharding import Mesh, PartitionSpec
    from jax.experimental.shard_map import shard_map
    from concourse import bass2jax

    nc, consts = _get_program()
    bass2jax.install_neuronx_cc_hook()

    part_name = nc.partition_id_tensor.name if nc.partition_id_tensor else None
    in_names, out_names, out_avals, zero_outs = [], [], [], []
    for alloc in nc.m.functions[0].allocations:
        if not isinstance(alloc, mybir.MemoryLocationSet):
            continue
        name = alloc.memorylocations[0].name
        if alloc.kind == "ExternalInput":
            if name != part_name:
                in_names.append(name)
        elif alloc.kind == "ExternalOutput":
            out_names.append(name)
            shape = tuple(alloc.tensor_shape)
            dtype = mybir.dt.np(alloc.dtype)
            out_avals.append(jax.core.ShapedArray(shape, dtype))
            zero_outs.append(np.zeros(shape, dtype))
    n_params = len(in_names)
    all_names = in_names + out_names
    if part_name is not None:
        all_names = all_names + [part_name]
    donate = tuple(range(n_params, n_params + len(out_names)))

    def _body(*args):
        operands = list(args)
        if part_name is not None:
            operands.append(bass2jax.partition_id_tensor())
        outs = bass2jax._bass_exec_p.bind(
            *operands,
            out_avals=tuple(out_avals),
            in_names=tuple(all_names),
            out_names=tuple(out_names),
            lowering_input_output_aliases=(),
            sim_require_finite=True,
            sim_require_nnan=True,
            nc=nc,
        )
        return tuple(outs)

    devices = jax.devices()[:N_CORES]
    mesh = Mesh(np.asarray(devices), ("core",))
    specs = (PartitionSpec("core"),) * (n_params + len(out_names))
    sharded = jax.jit(
        shard_map(_body, mesh=mesh, in_specs=specs,
                  out_specs=(PartitionSpec("core"),) * len(out_names),
                  check_rep=False),
        donate_argnums=donate, keep_unused=True,
    )
    runner = dict(fn=sharded, in_names=in_names, out_names=out_names,
                  zero_outs=zero_outs, consts=consts, nc=nc)
    _CACHE["runner"] = runner
    return runner


def _concat_inputs(q, k, runner):
    """Per-core input dict -> globally concatenated arrays (axis 0)."""
    consts = runner["consts"]
    arrs = []
    for name in runner["in_names"]:
        if name == "q":
            arrs.append(q)
        elif name == "k":
            arrs.append(k)
        else:
            c = consts[name]
            arrs.append(np.concatenate([c] * N_CORES, axis=0))
    return arrs


def kernel(q, k):
    q = np.ascontiguousarray(np.asarray(q, dtype=np.float32))
    k = np.ascontiguousarray(np.asarray(k, dtype=np.float32))
    assert q.shape == (BH_TOTAL, SEQ, D) and k.shape == (BH_TOTAL, SEQ, D)

    runner = _get_runner()
    # bh-shard across 8 cores: core c gets bh slice [4c, 4c+4). The global
    # concat layout [32, ...] already matches (shard_map splits axis 0).
    concat_in = _concat_inputs(q, k, runner)
    concat_zeros = [np.zeros((N_CORES * z.shape[0], *z.shape[1:]), z.dtype)
                    for z in runner["zero_outs"]]
    out_arrs = runner["fn"](*concat_in, *concat_zeros)
    out = np.asarray(out_arrs[0])          # [8*4, 128, 129]
    return np.ascontiguousarray(out.reshape(BH_TOTAL, NB, NJ))


# revision 39
# speedup vs baseline: 991.6560x; 991.6560x over previous
"""Trainium2 Bass kernel for CausalAttentionSortNet (bucketed causal sort-net scores).

Math (per bh slice; n=8192, bucket=64, nb=128 buckets, d=64):
  sq[i]  = cumavg(q)[64*i]            = (sum_{s<=64i} q[s]) / (64i+1)
  sk[j]  = sum_sigma cumavg(k)[64j+s] = H_j * Bk[j] + sum_s G[j,s] k[64j+s]
           where Bk[j] = sum of full buckets < j, H_j = sum_s 1/(64j+s+1),
           G[j,s] = sum_{s'>=s} 1/(64j+s'+1)
  R[i,jj] = (sq[i] . skp[jj]) / 8 ; skp = [0, sk[0..126]] padded front
  masked softmax over jj<=i, then keep strictly jj<i.

Layout trick: per bh, DMA q/k as [128 partitions, 4096] where partition
p = 4*jj + c holds seq rows 64*(32t+jj) + 16c + s (t in free dim) -> each
partition reads 4KB-contiguous HBM chunks. Bucket sums are then a PE
matmul with a 0/1 quarter-fold stationary (collapsing the 4 quarters c)
followed by a short DVE reduce over s (16 strided elems). Prefix-over-
buckets, transposes and the final 128x129 score matmul all run on PE.
"""

import numpy as np
from contextlib import ExitStack

import concourse.bass as bass
import concourse.tile as tile
from concourse.tile import add_dep_helper
from concourse import mybir
from concourse import bass_utils

# ---------------- problem constants (hardcoded per spec) ----------------
BH_TOTAL = 32
N_CORES = 8
BH = BH_TOTAL // N_CORES          # 4 bh slices per core
SEQ = 8192
D = 64
BUCKET = 64
NB = SEQ // BUCKET                # 128 buckets
NJ = NB + 1                       # 129 output cols
NEG = -1e30

_F32 = mybir.dt.float32


def _host_constants():
    inv = 1.0 / np.arange(1, SEQ + 1, dtype=np.float64)          # 1/(t+1)
    invb = inv.reshape(NB, BUCKET)                               # [j, s]
    H = invb.sum(axis=1)                                         # [128]
    # suffix sums within bucket: G[j, s] = sum_{s'>=s} inv[j, s']
    G = np.cumsum(invb[:, ::-1], axis=1)[:, ::-1]                # [128, 64]

    i_idx = np.arange(NB)
    c8 = 1.0 / (8.0 * (BUCKET * i_idx + 1))                      # c_i/8
    j_col = i_idx[:, None]
    i_row = i_idx[None, :]
    pmq = np.where(j_col < i_row, c8[None, :], 0.0)              # [j, i]
    pmk = np.where(j_col < i_row, H[None, :], 0.0)               # [j, j2]

    # bucket-contiguous: gw[j, 64*s + d] = G[j, s] (broadcast over d)
    gw = np.repeat(G[:, :, None], D, axis=2).reshape(128, 4096)

    ident = np.eye(128)

    jj_col = np.arange(NJ)[None, :]
    i_rows = np.arange(NB)[:, None]
    maskneg = np.where(jj_col <= i_rows, 0.0, NEG)               # [128, 129]
    maskstrict = (jj_col < i_rows).astype(np.float64)            # [128, 129]

    f = np.float32
    cpack = np.concatenate([
        pmq, pmk, c8.reshape(128, 1), ident, maskneg, maskstrict,
    ], axis=1)
    return dict(gw=gw.astype(f), cpack=cpack.astype(f))


def _build_program():
    nc = bass.Bass("TRN2", target_bir_lowering=False, debug=False)

    q_t = nc.dram_tensor("q", [BH, SEQ, D], _F32, kind="ExternalInput")
    k_t = nc.dram_tensor("k", [BH, SEQ, D], _F32, kind="ExternalInput")
    gw_t = nc.dram_tensor("gw", [128, 4096], _F32, kind="ExternalInput")
    cp_t = nc.dram_tensor("cpack", [128, 643], _F32, kind="ExternalInput")
    out_t = nc.dram_tensor("out", [BH, NB, NJ], _F32, kind="ExternalOutput")

    with tile.TileContext(nc) as tc, ExitStack() as ctx:
        _body(ctx, tc, q_t.ap(), k_t.ap(), out_t.ap(), gw_t.ap(), cp_t.ap())
    _split_matmul_waits(nc)
    return nc


_NO_SPLIT = ()


def _split_matmul_waits(nc):
    """This walrus build rejects compute instructions carrying more than one
    sync wait. Moving the waits onto single-wait NoOps placed immediately
    before the instruction in the same engine queue is semantically
    identical: the sequencer executes waits in queue order before
    dispatching."""
    n = 0
    for f in nc.m.functions:
        for b in f.blocks:
            insts = list(b.instructions)
            out = []
            changed = False
            for i in insts:
                si = getattr(i, "sync_info", None)
                if (si is not None and len(si.on_wait) > 1
                        and type(i).__name__ not in _NO_SPLIT
                        and i.is_executable()):
                    n += 1
                    changed = True
                    for wi, w in enumerate(si.on_wait):
                        nop = mybir.InstNoOp(
                            name=f"{i.name}-wsplit{wi}", ins=[], outs=[])
                        nop.engine = i.engine
                        nop.sync_info = mybir.SyncInfo(on_wait=[w], on_update=[])
                        out.append(nop)
                    i.sync_info = mybir.SyncInfo(
                        on_wait=[], on_update=list(si.on_update))
                out.append(i)
            if changed:
                b.instructions = out
    return n


def _body(ctx, tc, q, k, out, gw_d, cp_d):
    nc = tc.nc
    cpool = ctx.enter_context(tc.tile_pool(name="consts", bufs=1))
    dpool = ctx.enter_context(tc.tile_pool(name="data", bufs=3))
    spool = ctx.enter_context(tc.tile_pool(name="small", bufs=2))
    ppool = ctx.enter_context(tc.tile_pool(name="psum", bufs=2, space="PSUM"))

    # ---- resident constants: gw first (gates the gpsimd multiplies),
    # then everything else in a single packed DMA ----
    gw = cpool.tile([128, 4096], _F32, tag="gw")
    nc.sync.dma_start(gw[:], gw_d)
    cpk = cpool.tile([128, 643], _F32, tag="cpack")
    nc.sync.dma_start(cpk[:], cp_d)
    pmq = cpk[:, 0:128]
    pmk = cpk[:, 128:256]
    cq8 = cpk[:, 256:257]
    ident = cpk[:, 257:385]
    maskneg = cpk[:, 385:514]
    maskstrict = cpk[:, 514:643]

    def stage_load(bh):
        # bucket-contiguous: partition j holds rows [64j, 64j+64) = 16KB
        kt = dpool.tile([128, 4096], _F32, tag="kt", bufs=4)
        nc.sync.dma_start(kt[:], k[bh].rearrange("(j r) d -> j (r d)", r=64))
        qt = dpool.tile([128, 4096], _F32, tag="qt", bufs=4)
        nc.sync.dma_start(qt[:], q[bh].rearrange("(j r) d -> j (r d)", r=64))
        # kw = kt * G (broadcast over d) on GPSIMD, chunked for pipelining
        kw = dpool.tile([128, 4096], _F32, tag="kw", bufs=2)
        for c in range(2):
            sl = slice(2048 * c, 2048 * (c + 1))
            nc.gpsimd.tensor_mul(kw[:, sl], kt[:, sl], gw[:, sl])

        def _v(tl):
            return tl[:].rearrange("j (s d) -> j d s", s=64, d=64)

        kb = spool.tile([128, D], _F32, tag="kb")
        nc.vector.reduce_sum(kb[:], _v(kt), axis=mybir.AxisListType.X)
        qb = spool.tile([128, D], _F32, tag="qb")
        nc.vector.reduce_sum(qb[:], _v(qt), axis=mybir.AxisListType.X)
        return dict(kt=kt, qt=qt, kw=kw, kb=kb, qb=qb, v=_v)

    def stage_finish(bh, st):
        kg = spool.tile([128, D], _F32, tag="kg")
        nc.vector.reduce_sum(kg[:], st["v"](st["kw"]), axis=mybir.AxisListType.X)
        qf = st["qt"][:, 0:D]            # q[64j, :] = sigma=0 slice
        qb, kb = st["qb"], st["kb"]

        # prefix over buckets (PE), then combine
        paccs = ppool.tile([128, 128], _F32, tag="paccs")
        nc.tensor.matmul(paccs[:, 0:64], pmq, qb[:], start=True, stop=True)
        nc.tensor.matmul(paccs[:, 64:128], pmk, kb[:], start=True, stop=True)

        sq = spool.tile([128, D], _F32, tag="sq")
        nc.vector.scalar_tensor_tensor(sq[:], qf, cq8, paccs[:, 0:64],
                                       op0=mybir.AluOpType.mult,
                                       op1=mybir.AluOpType.add)
        sk = spool.tile([128, D], _F32, tag="sk")
        nc.vector.tensor_add(sk[:], paccs[:, 64:128], kg[:])

        # transposes + score matmul
        ptr = ppool.tile([128, 512], _F32, tag="ptr")
        nc.tensor.transpose(ptr[0:64, 256:384], sq[:], ident)
        sqT = spool.tile([64, 128], _F32, tag="sqT")
        nc.scalar.copy(sqT[:], ptr[0:64, 256:384])
        nc.tensor.transpose(ptr[0:64, 384:512], sk[:], ident)
        skpT = spool.tile([64, NJ + 3], _F32, tag="skpT")
        nc.vector.memset(skpT[:, 0:1], 0.0)
        nc.scalar.copy(skpT[:, 1:129], ptr[0:64, 384:512])

        nc.tensor.matmul(ptr[:, 0:NJ], sqT[:], skpT[:, 0:NJ], start=True, stop=True)

        # masked softmax
        Rm = spool.tile([128, NJ], _F32, tag="Rm")
        nc.vector.tensor_add(Rm[:], ptr[:, 0:NJ], maskneg)
        nm = spool.tile([128, 1], _F32, tag="nm")
        nc.vector.reduce_max(nm[:], Rm[:], axis=mybir.AxisListType.X, negate=True)
        e = spool.tile([128, NJ], _F32, tag="e")
        den = spool.tile([128, 1], _F32, tag="den")
        nc.scalar.activation(e[:], Rm[:], mybir.ActivationFunctionType.Exp,
                             bias=nm[:], scale=1.0, accum_out=den[:])
        rden = spool.tile([128, 1], _F32, tag="rden")
        nc.vector.reciprocal(rden[:], den[:])
        outb = spool.tile([128, NJ], _F32, tag="outb")
        nc.vector.scalar_tensor_tensor(outb[:], e[:], rden[:], maskstrict,
                                       op0=mybir.AluOpType.mult,
                                       op1=mybir.AluOpType.mult)
        nc.sync.dma_start(out[bh], outb[:])

    # software pipeline: bh's kg-reduce and tail phases are created after
    # bh+1's loads/plain-reduces so the DVE FIFO is never head-blocked on
    # the GPSIMD multiply chain.
    pend = None
    for bh in range(BH):
        st = stage_load(bh)
        if pend is not None:
            stage_finish(bh - 1, pend)
        pend = st
    stage_finish(BH - 1, pend)


_CACHE = {}


def _get_program():
    if "nc" not in _CACHE:
        _CACHE["nc"] = _build_program()
        _CACHE["consts"] = _host_constants()
    return _CACHE["nc"], _CACHE["consts"]


def _get_runner():
    """Build the sharded PJRT callable once and cache it (mirrors
    bass2jax.run_bass_via_pjrt but reuses the jitted function across
    calls)."""
    if "runner" in _CACHE:
        return _CACHE["runner"]
    import jax
    from jax.sharding import Mesh, PartitionSpec
    from jax.experimental.shard_map import shard_map
    from concourse import bass2jax

    nc, consts = _get_program()
    bass2jax.install_neuronx_cc_hook()

    part_name = nc.partition_id_tensor.name if nc.partition_id_tensor else None
    in_names, out_names, out_avals, zero_outs = [], [], [], []
    for alloc in nc.m.functions[0].allocations:
        if not isinstance(alloc, mybir.MemoryLocationSet):
            continue
        name = alloc.memorylocations[0].name
        if alloc.kind == "ExternalInput":
            if name != part_name:
                in_names.append(name)
        elif alloc.kind == "ExternalOutput":
            out_names.append(name)
            shape = tuple(alloc.tensor_shape)
            dtype = mybir.dt.np(alloc.dtype)
            out_avals.append(jax.core.ShapedArray(shape, dtype))
            zero_outs.append(np.zeros(shape, dtype))
    n_params = len(in_names)
    all_names = in_names + out_names
    if part_name is not None:
        all_names = all_names + [part_name]
    donate = tuple(range(n_params, n_params + len(out_names)))

    def _body(*args):
        operands = list(args)
        if part_name is not None:
            operands.append(bass2jax.partition_id_tensor())
        outs = bass2jax._bass_exec_p.bind(
            *operands,
            out_avals=tuple(out_avals),
            in_names=tuple(all_names),
            out_names=tuple(out_names),
            lowering_input_output_aliases=(),
            sim_require_finite=True,
            sim_require_nnan=True,
            nc=nc,
        )
        return tuple(outs)

    devices = jax.devices()[:N_CORES]
    mesh = Mesh(np.asarray(devices), ("core",))
    specs = (PartitionSpec("core"),) * (n_params + len(out_names))
    sharded = jax.jit(
        shard_map(_body, mesh=mesh, in_specs=specs,
                  out_specs=(PartitionSpec("core"),) * len(out_names),
                  check_rep=False),
        donate_argnums=donate, keep_unused=True,
    )
    runner = dict(fn=sharded, in_names=in_names, out_names=out_names,
                  zero_outs=zero_outs, consts=consts, nc=nc)
    _CACHE["runner"] = runner
    return runner


def _concat_inputs(q, k, runner):
    """Per-core input dict -> globally concatenated arrays (axis 0)."""
    consts = runner["consts"]
    arrs = []
    for name in runner["in_names"]:
        if name == "q":
            arrs.append(q)
        elif name == "k":
            arrs.append(k)
        else:
            c = consts[name]
            arrs.append(np.concatenate([c] * N_CORES, axis=0))
    return arrs


def kernel(q, k):
    q = np.ascontiguousarray(np.asarray(q, dtype=np.float32))
    k = np.ascontiguousarray(np.asarray(k, dtype=np.float32))
    assert q.shape == (BH_TOTAL, SEQ, D) and k.shape == (BH_TOTAL, SEQ, D)

    runner = _get_runner()
    # bh-shard across 8 cores: core c gets bh slice [4c, 4c+4). The global
    # concat layout [32, ...] already matches (shard_map splits axis 0).
    concat_in = _concat_inputs(q, k, runner)
    concat_zeros = [np.zeros((N_CORES * z.shape[0], *z.shape[1:]), z.dtype)
                    for z in runner["zero_outs"]]
    out_arrs = runner["fn"](*concat_in, *concat_zeros)
    out = np.asarray(out_arrs[0])          # [8*4, 128, 129]
    return np.ascontiguousarray(out.reshape(BH_TOTAL, NB, NJ))


# revision 40
# speedup vs baseline: 1066.5097x; 1.0755x over previous
"""Trainium2 Bass kernel for CausalAttentionSortNet (bucketed causal sort-net scores).

Math (per bh slice; n=8192, bucket=64, nb=128 buckets, d=64):
  sq[i]  = cumavg(q)[64*i]            = (sum_{s<=64i} q[s]) / (64i+1)
  sk[j]  = sum_sigma cumavg(k)[64j+s] = H_j * Bk[j] + sum_s G[j,s] k[64j+s]
           where Bk[j] = sum of full buckets < j, H_j = sum_s 1/(64j+s+1),
           G[j,s] = sum_{s'>=s} 1/(64j+s'+1)
  R[i,jj] = (sq[i] . skp[jj]) / 8 ; skp = [0, sk[0..126]] padded front
  masked softmax over jj<=i, then keep strictly jj<i.

Layout trick: per bh, DMA q/k as [128 partitions, 4096] where partition
p = 4*jj + c holds seq rows 64*(32t+jj) + 16c + s (t in free dim) -> each
partition reads 4KB-contiguous HBM chunks. Bucket sums are then a PE
matmul with a 0/1 quarter-fold stationary (collapsing the 4 quarters c)
followed by a short DVE reduce over s (16 strided elems). Prefix-over-
buckets, transposes and the final 128x129 score matmul all run on PE.
"""

import numpy as np
from contextlib import ExitStack

import concourse.bass as bass
import concourse.tile as tile
from concourse.tile import add_dep_helper
from concourse import mybir
from concourse import bass_utils

# ---------------- problem constants (hardcoded per spec) ----------------
BH_TOTAL = 32
N_CORES = 8
BH = BH_TOTAL // N_CORES          # 4 bh slices per core
SEQ = 8192
D = 64
BUCKET = 64
NB = SEQ // BUCKET                # 128 buckets
NJ = NB + 1                       # 129 output cols
NEG = -1e30

_F32 = mybir.dt.float32


def _host_constants():
    inv = 1.0 / np.arange(1, SEQ + 1, dtype=np.float64)          # 1/(t+1)
    invb = inv.reshape(NB, BUCKET)                               # [j, s]
    H = invb.sum(axis=1)                                         # [128]
    # suffix sums within bucket: G[j, s] = sum_{s'>=s} inv[j, s']
    G = np.cumsum(invb[:, ::-1], axis=1)[:, ::-1]                # [128, 64]

    i_idx = np.arange(NB)
    c8 = 1.0 / (8.0 * (BUCKET * i_idx + 1))                      # c_i/8
    j_col = i_idx[:, None]
    i_row = i_idx[None, :]
    pmq = np.where(j_col < i_row, c8[None, :], 0.0)              # [j, i]
    pmk = np.where(j_col < i_row, H[None, :], 0.0)               # [j, j2]

    # bucket-contiguous: gw[j, 64*s + d] = G[j, s] (broadcast over d)
    gw = np.repeat(G[:, :, None], D, axis=2).reshape(128, 4096)

    ident = np.eye(128)

    jj_col = np.arange(NJ)[None, :]
    i_rows = np.arange(NB)[:, None]
    maskneg = np.where(jj_col <= i_rows, 0.0, NEG)               # [128, 129]
    maskstrict = (jj_col < i_rows).astype(np.float64)            # [128, 129]

    f = np.float32
    cpack = np.concatenate([
        pmq, pmk, c8.reshape(128, 1), ident, maskneg, maskstrict,
    ], axis=1)
    return dict(gw=gw.astype(f), cpack=cpack.astype(f))


def _build_program():
    nc = bass.Bass("TRN2", target_bir_lowering=False, debug=False)

    q_t = nc.dram_tensor("q", [BH, SEQ, D], _F32, kind="ExternalInput")
    k_t = nc.dram_tensor("k", [BH, SEQ, D], _F32, kind="ExternalInput")
    gw_t = nc.dram_tensor("gw", [128, 4096], _F32, kind="ExternalInput")
    cp_t = nc.dram_tensor("cpack", [128, 643], _F32, kind="ExternalInput")
    out_t = nc.dram_tensor("out", [BH, NB, NJ], _F32, kind="ExternalOutput")

    with tile.TileContext(nc) as tc, ExitStack() as ctx:
        _body(ctx, tc, q_t.ap(), k_t.ap(), out_t.ap(), gw_t.ap(), cp_t.ap())
    _split_matmul_waits(nc)
    return nc


_NO_SPLIT = ()


def _split_matmul_waits(nc):
    """This walrus build rejects compute instructions carrying more than one
    sync wait. Moving the waits onto single-wait NoOps placed immediately
    before the instruction in the same engine queue is semantically
    identical: the sequencer executes waits in queue order before
    dispatching."""
    n = 0
    for f in nc.m.functions:
        for b in f.blocks:
            insts = list(b.instructions)
            out = []
            changed = False
            for i in insts:
                si = getattr(i, "sync_info", None)
                if (si is not None and len(si.on_wait) > 1
                        and type(i).__name__ not in _NO_SPLIT
                        and i.is_executable()):
                    n += 1
                    changed = True
                    for wi, w in enumerate(si.on_wait):
                        nop = mybir.InstNoOp(
                            name=f"{i.name}-wsplit{wi}", ins=[], outs=[])
                        nop.engine = i.engine
                        nop.sync_info = mybir.SyncInfo(on_wait=[w], on_update=[])
                        out.append(nop)
                    i.sync_info = mybir.SyncInfo(
                        on_wait=[], on_update=list(si.on_update))
                out.append(i)
            if changed:
                b.instructions = out
    return n


def _body(ctx, tc, q, k, out, gw_d, cp_d):
    nc = tc.nc
    cpool = ctx.enter_context(tc.tile_pool(name="consts", bufs=1))
    dpool = ctx.enter_context(tc.tile_pool(name="data", bufs=3))
    spool = ctx.enter_context(tc.tile_pool(name="small", bufs=2))
    ppool = ctx.enter_context(tc.tile_pool(name="psum", bufs=2, space="PSUM"))

    # ---- resident constants: gw first (gates the gpsimd multiplies),
    # then everything else in a single packed DMA ----
    gw = cpool.tile([128, 4096], _F32, tag="gw")
    nc.sync.dma_start(gw[:], gw_d)
    cpk = cpool.tile([128, 643], _F32, tag="cpack")
    nc.sync.dma_start(cpk[:], cp_d)
    pmq = cpk[:, 0:128]
    pmk = cpk[:, 128:256]
    cq8 = cpk[:, 256:257]
    ident = cpk[:, 257:385]
    maskneg = cpk[:, 385:514]
    maskstrict = cpk[:, 514:643]

    def stage_load(bh):
        # bucket-contiguous: partition j holds rows [64j, 64j+64) = 16KB
        kt = dpool.tile([128, 4096], _F32, tag="kt", bufs=3)
        nc.sync.dma_start(kt[:], k[bh].rearrange("(j r) d -> j (r d)", r=64))
        qt = dpool.tile([128, 4096], _F32, tag="qt", bufs=3)
        nc.sync.dma_start(qt[:], q[bh].rearrange("(j r) d -> j (r d)", r=64))
        # kw = kt * G (broadcast over d) on GPSIMD, chunked for pipelining
        kw = dpool.tile([128, 4096], _F32, tag="kw", bufs=2)
        for c in range(2):
            sl = slice(2048 * c, 2048 * (c + 1))
            nc.gpsimd.tensor_mul(kw[:, sl], kt[:, sl], gw[:, sl])

        def _v(tl):
            return tl[:].rearrange("j (s d) -> j d s", s=64, d=64)

        kb = spool.tile([128, D], _F32, tag="kb")
        nc.vector.reduce_sum(kb[:], _v(kt), axis=mybir.AxisListType.X)
        # qb: gpsimd pre-folds the two sigma-halves (contiguous add), DVE
        # finishes with a half-length strided reduce — rebalances DVE/Pool.
        t1q = dpool.tile([128, 2048], _F32, tag="t1q", bufs=2)
        nc.gpsimd.tensor_add(t1q[:], qt[:, 0:2048], qt[:, 2048:4096])
        qb = spool.tile([128, D], _F32, tag="qb")
        nc.vector.reduce_sum(
            qb[:], t1q[:].rearrange("j (s d) -> j d s", s=32, d=64),
            axis=mybir.AxisListType.X)
        return dict(kt=kt, qt=qt, kw=kw, kb=kb, qb=qb, v=_v)

    def stage_finish(bh, st):
        kg = spool.tile([128, D], _F32, tag="kg")
        nc.vector.reduce_sum(kg[:], st["v"](st["kw"]), axis=mybir.AxisListType.X)
        qf = st["qt"][:, 0:D]            # q[64j, :] = sigma=0 slice
        qb, kb = st["qb"], st["kb"]

        # prefix over buckets (PE), then combine
        paccs = ppool.tile([128, 128], _F32, tag="paccs")
        nc.tensor.matmul(paccs[:, 0:64], pmq, qb[:], start=True, stop=True)
        nc.tensor.matmul(paccs[:, 64:128], pmk, kb[:], start=True, stop=True)

        sq = spool.tile([128, D], _F32, tag="sq")
        nc.vector.scalar_tensor_tensor(sq[:], qf, cq8, paccs[:, 0:64],
                                       op0=mybir.AluOpType.mult,
                                       op1=mybir.AluOpType.add)
        sk = spool.tile([128, D], _F32, tag="sk")
        nc.vector.tensor_add(sk[:], paccs[:, 64:128], kg[:])

        # transposes + score matmul
        ptr = ppool.tile([128, 512], _F32, tag="ptr")
        nc.tensor.transpose(ptr[0:64, 256:384], sq[:], ident)
        sqT = spool.tile([64, 128], _F32, tag="sqT")
        nc.scalar.copy(sqT[:], ptr[0:64, 256:384])
        nc.tensor.transpose(ptr[0:64, 384:512], sk[:], ident)
        skpT = spool.tile([64, NJ + 3], _F32, tag="skpT")
        nc.vector.memset(skpT[:, 0:1], 0.0)
        nc.scalar.copy(skpT[:, 1:129], ptr[0:64, 384:512])

        nc.tensor.matmul(ptr[:, 0:NJ], sqT[:], skpT[:, 0:NJ], start=True, stop=True)

        # masked softmax
        Rm = spool.tile([128, NJ], _F32, tag="Rm")
        nc.vector.tensor_add(Rm[:], ptr[:, 0:NJ], maskneg)
        nm = spool.tile([128, 1], _F32, tag="nm")
        nc.vector.reduce_max(nm[:], Rm[:], axis=mybir.AxisListType.X, negate=True)
        e = spool.tile([128, NJ], _F32, tag="e")
        den = spool.tile([128, 1], _F32, tag="den")
        nc.scalar.activation(e[:], Rm[:], mybir.ActivationFunctionType.Exp,
                             bias=nm[:], scale=1.0, accum_out=den[:])
        rden = spool.tile([128, 1], _F32, tag="rden")
        nc.vector.reciprocal(rden[:], den[:])
        outb = spool.tile([128, NJ], _F32, tag="outb")
        nc.vector.scalar_tensor_tensor(outb[:], e[:], rden[:], maskstrict,
                                       op0=mybir.AluOpType.mult,
                                       op1=mybir.AluOpType.mult)
        nc.sync.dma_start(out[bh], outb[:])

    # software pipeline: bh's kg-reduce and tail phases are created after
    # bh+1's loads/plain-reduces so the DVE FIFO is never head-blocked on
    # the GPSIMD multiply chain.
    pend = None
    for bh in range(BH):
        st = stage_load(bh)
        if pend is not None:
            stage_finish(bh - 1, pend)
        pend = st
    stage_finish(BH - 1, pend)


_CACHE = {}


def _get_program():
    if "nc" not in _CACHE:
        _CACHE["nc"] = _build_program()
        _CACHE["consts"] = _host_constants()
    return _CACHE["nc"], _CACHE["consts"]


def _get_runner():
    """Build the sharded PJRT callable once and cache it (mirrors
    bass2jax.run_bass_via_pjrt but reuses the jitted function across
    calls)."""
    if "runner" in _CACHE:
        return _CACHE["runner"]
    import jax
    from jax.sharding import Mesh, PartitionSpec
    from jax.experimental.shard_map import shard_map
    from concourse import bass2jax

    nc, consts = _get_program()
    bass2jax.install_neuronx_cc_hook()

    part_name = nc.partition_id_tensor.name if nc.partition_id_tensor else None
    in_names, out_names, out_avals, zero_outs = [], [], [], []
    for alloc in nc.m.functions[0].allocations:
        if not isinstance(alloc, mybir.MemoryLocationSet):
            continue
        name = alloc.memorylocations[0].name
        if alloc.kind == "ExternalInput":
            if name != part_name:
                in_names.append(name)
        elif alloc.kind == "ExternalOutput":
            out_names.append(name)
            shape = tuple(alloc.tensor_shape)
            dtype = mybir.dt.np(alloc.dtype)
            out_avals.append(jax.core.ShapedArray(shape, dtype))
            zero_outs.append(np.zeros(shape, dtype))
    n_params = len(in_names)
    all_names = in_names + out_names
    if part_name is not None:
        all_names = all_names + [part_name]
    donate = tuple(range(n_params, n_params + len(out_names)))

    def _body(*args):
        operands = list(args)
        if part_name is not None:
            operands.append(bass2jax.partition_id_tensor())
        outs = bass2jax._bass_exec_p.bind(
            *operands,
            out_avals=tuple(out_avals),
            in_names=tuple(all_names),
            out_names=tuple(out_names),
            lowering_input_output_aliases=(),
            sim_require_finite=True,
            sim_require_nnan=True,
            nc=nc,
        )
        return tuple(outs)

    devices = jax.devices()[:N_CORES]
    mesh = Mesh(np.asarray(devices), ("core",))
    specs = (PartitionSpec("core"),) * (n_params + len(out_names))
    sharded = jax.jit(
        shard_map(_body, mesh=mesh, in_specs=specs,
                  out_specs=(PartitionSpec("core"),) * len(out_names),
                  check_rep=False),
        donate_argnums=donate, keep_unused=True,
    )
    runner = dict(fn=sharded, in_names=in_names, out_names=out_names,
                  zero_outs=zero_outs, consts=consts, nc=nc)
    _CACHE["runner"] = runner
    return runner


def _concat_inputs(q, k, runner):
    """Per-core input dict -> globally concatenated arrays (axis 0)."""
    consts = runner["consts"]
    arrs = []
    for name in runner["in_names"]:
        if name == "q":
            arrs.append(q)
        elif name == "k":
            arrs.append(k)
        else:
            c = consts[name]
            arrs.append(np.concatenate([c] * N_CORES, axis=0))
    return arrs


def kernel(q, k):
    q = np.ascontiguousarray(np.asarray(q, dtype=np.float32))
    k = np.ascontiguousarray(np.asarray(k, dtype=np.float32))
    assert q.shape == (BH_TOTAL, SEQ, D) and k.shape == (BH_TOTAL, SEQ, D)

    runner = _get_runner()
    # bh-shard across 8 cores: core c gets bh slice [4c, 4c+4). The global
    # concat layout [32, ...] already matches (shard_map splits axis 0).
    concat_in = _concat_inputs(q, k, runner)
    concat_zeros = [np.zeros((N_CORES * z.shape[0], *z.shape[1:]), z.dtype)
                    for z in runner["zero_outs"]]
    out_arrs = runner["fn"](*concat_in, *concat_zeros)
    out = np.asarray(out_arrs[0])          # [8*4, 128, 129]
    return np.ascontiguousarray(out.reshape(BH_TOTAL, NB, NJ))


# revision 43
# speedup vs baseline: 1106.6110x; 1.0376x over previous
"""Trainium2 Bass kernel for CausalAttentionSortNet (bucketed causal sort-net scores).

Math (per bh slice; n=8192, bucket=64, nb=128 buckets, d=64):
  sq[i]  = cumavg(q)[64*i]            = (sum_{s<=64i} q[s]) / (64i+1)
  sk[j]  = sum_sigma cumavg(k)[64j+s] = H_j * Bk[j] + sum_s G[j,s] k[64j+s]
           where Bk[j] = sum of full buckets < j, H_j = sum_s 1/(64j+s+1),
           G[j,s] = sum_{s'>=s} 1/(64j+s'+1)
  R[i,jj] = (sq[i] . skp[jj]) / 8 ; skp = [0, sk[0..126]] padded front
  masked softmax over jj<=i, then keep strictly jj<i.

Layout trick: per bh, DMA q/k as [128 partitions, 4096] where partition
p = 4*jj + c holds seq rows 64*(32t+jj) + 16c + s (t in free dim) -> each
partition reads 4KB-contiguous HBM chunks. Bucket sums are then a PE
matmul with a 0/1 quarter-fold stationary (collapsing the 4 quarters c)
followed by a short DVE reduce over s (16 strided elems). Prefix-over-
buckets, transposes and the final 128x129 score matmul all run on PE.
"""

import numpy as np
from contextlib import ExitStack

import concourse.bass as bass
import concourse.tile as tile
from concourse.tile import add_dep_helper
from concourse import mybir
from concourse import bass_utils

# ---------------- problem constants (hardcoded per spec) ----------------
BH_TOTAL = 32
N_CORES = 8
BH = BH_TOTAL // N_CORES          # 4 bh slices per core
SEQ = 8192
D = 64
BUCKET = 64
NB = SEQ // BUCKET                # 128 buckets
NJ = NB + 1                       # 129 output cols
NEG = -1e30

_F32 = mybir.dt.float32


def _host_constants():
    inv = 1.0 / np.arange(1, SEQ + 1, dtype=np.float64)          # 1/(t+1)
    invb = inv.reshape(NB, BUCKET)                               # [j, s]
    H = invb.sum(axis=1)                                         # [128]
    # suffix sums within bucket: G[j, s] = sum_{s'>=s} inv[j, s']
    G = np.cumsum(invb[:, ::-1], axis=1)[:, ::-1]                # [128, 64]

    i_idx = np.arange(NB)
    c8 = 1.0 / (8.0 * (BUCKET * i_idx + 1))                      # c_i/8
    j_col = i_idx[:, None]
    i_row = i_idx[None, :]
    pmq = np.where(j_col < i_row, c8[None, :], 0.0)              # [j, i]
    pmk = np.where(j_col < i_row, H[None, :], 0.0)               # [j, j2]

    # bucket-contiguous: gw[j, 64*s + d] = G[j, s] (broadcast over d)
    gw = np.repeat(G[:, :, None], D, axis=2).reshape(128, 4096)

    ident = np.eye(128)

    jj_col = np.arange(NJ)[None, :]
    i_rows = np.arange(NB)[:, None]
    maskneg = np.where(jj_col <= i_rows, 0.0, NEG)               # [128, 129]
    maskstrict = (jj_col < i_rows).astype(np.float64)            # [128, 129]

    f = np.float32
    cpack = np.concatenate([
        pmq, pmk, c8.reshape(128, 1), ident, maskneg, maskstrict,
    ], axis=1)
    return dict(gw=gw.astype(f), cpack=cpack.astype(f))


def _build_program():
    nc = bass.Bass("TRN2", target_bir_lowering=False, debug=False)

    q_t = nc.dram_tensor("q", [BH, SEQ, D], _F32, kind="ExternalInput")
    k_t = nc.dram_tensor("k", [BH, SEQ, D], _F32, kind="ExternalInput")
    gw_t = nc.dram_tensor("gw", [128, 4096], _F32, kind="ExternalInput")
    cp_t = nc.dram_tensor("cpack", [128, 643], _F32, kind="ExternalInput")
    out_t = nc.dram_tensor("out", [BH, NB, NJ], _F32, kind="ExternalOutput")

    with tile.TileContext(nc) as tc, ExitStack() as ctx:
        _body(ctx, tc, q_t.ap(), k_t.ap(), out_t.ap(), gw_t.ap(), cp_t.ap())
    _split_matmul_waits(nc)
    return nc


_NO_SPLIT = ()


def _split_matmul_waits(nc):
    """This walrus build rejects compute instructions carrying more than one
    sync wait. Moving the waits onto single-wait NoOps placed immediately
    before the instruction in the same engine queue is semantically
    identical: the sequencer executes waits in queue order before
    dispatching."""
    n = 0
    for f in nc.m.functions:
        for b in f.blocks:
            insts = list(b.instructions)
            out = []
            changed = False
            for i in insts:
                si = getattr(i, "sync_info", None)
                if (si is not None and len(si.on_wait) > 1
                        and type(i).__name__ not in _NO_SPLIT
                        and i.is_executable()):
                    n += 1
                    changed = True
                    for wi, w in enumerate(si.on_wait):
                        nop = mybir.InstNoOp(
                            name=f"{i.name}-wsplit{wi}", ins=[], outs=[])
                        nop.engine = i.engine
                        nop.sync_info = mybir.SyncInfo(on_wait=[w], on_update=[])
                        out.append(nop)
                    i.sync_info = mybir.SyncInfo(
                        on_wait=[], on_update=list(si.on_update))
                out.append(i)
            if changed:
                b.instructions = out
    return n


def _body(ctx, tc, q, k, out, gw_d, cp_d):
    nc = tc.nc
    cpool = ctx.enter_context(tc.tile_pool(name="consts", bufs=1))
    dpool = ctx.enter_context(tc.tile_pool(name="data", bufs=3))
    spool = ctx.enter_context(tc.tile_pool(name="small", bufs=2))
    ppool = ctx.enter_context(tc.tile_pool(name="psum", bufs=2, space="PSUM"))

    # ---- resident constants: gw first (gates the gpsimd multiplies),
    # then everything else in a single packed DMA ----
    gw = cpool.tile([128, 4096], _F32, tag="gw")
    nc.sync.dma_start(gw[:, 0:2048], gw_d[:, 0:2048])
    nc.sync.dma_start(gw[:, 2048:4096], gw_d[:, 2048:4096])
    cpk = cpool.tile([128, 643], _F32, tag="cpack")
    nc.sync.dma_start(cpk[:], cp_d)
    pmq = cpk[:, 0:128]
    pmk = cpk[:, 128:256]
    cq8 = cpk[:, 256:257]
    ident = cpk[:, 257:385]
    maskneg = cpk[:, 385:514]
    maskstrict = cpk[:, 514:643]

    def stage_load(bh):
        # bucket-contiguous: partition j holds rows [64j, 64j+64) = 16KB
        kt = dpool.tile([128, 4096], _F32, tag="kt", bufs=3)
        ksrc = k[bh].rearrange("(j r) d -> j (r d)", r=64)
        nc.sync.dma_start(kt[:, 0:2048], ksrc[:, 0:2048])
        nc.sync.dma_start(kt[:, 2048:4096], ksrc[:, 2048:4096])
        qt = dpool.tile([128, 4096], _F32, tag="qt", bufs=3)
        nc.sync.dma_start(qt[:], q[bh].rearrange("(j r) d -> j (r d)", r=64))
        # kw = kt * G (broadcast over d) on GPSIMD, chunked for pipelining
        kw = dpool.tile([128, 4096], _F32, tag="kw", bufs=2)
        for c in range(2):
            sl = slice(2048 * c, 2048 * (c + 1))
            nc.gpsimd.tensor_mul(kw[:, sl], kt[:, sl], gw[:, sl])

        def _v(tl):
            return tl[:].rearrange("j (s d) -> j d s", s=64, d=64)

        kb = spool.tile([128, D], _F32, tag="kb")
        nc.vector.reduce_sum(kb[:], _v(kt), axis=mybir.AxisListType.X)
        # qb: gpsimd pre-folds the two sigma-halves (contiguous add), DVE
        # finishes with a half-length strided reduce — rebalances DVE/Pool.
        t1q = dpool.tile([128, 2048], _F32, tag="t1q", bufs=2)
        nc.gpsimd.tensor_add(t1q[:], qt[:, 0:2048], qt[:, 2048:4096])
        qb = spool.tile([128, D], _F32, tag="qb")
        nc.vector.reduce_sum(
            qb[:], t1q[:].rearrange("j (s d) -> j d s", s=32, d=64),
            axis=mybir.AxisListType.X)
        return dict(kt=kt, qt=qt, kw=kw, kb=kb, qb=qb, v=_v)

    def stage_finish(bh, st):
        kg = spool.tile([128, D], _F32, tag="kg")
        nc.vector.reduce_sum(kg[:], st["v"](st["kw"]), axis=mybir.AxisListType.X)
        qf = st["qt"][:, 0:D]            # q[64j, :] = sigma=0 slice
        qb, kb = st["qb"], st["kb"]

        # prefix over buckets (PE), then combine
        paccs = ppool.tile([128, 128], _F32, tag="paccs")
        nc.tensor.matmul(paccs[:, 0:64], pmq, qb[:], start=True, stop=True)
        nc.tensor.matmul(paccs[:, 64:128], pmk, kb[:], start=True, stop=True)

        sq = spool.tile([128, D], _F32, tag="sq")
        nc.vector.scalar_tensor_tensor(sq[:], qf, cq8, paccs[:, 0:64],
                                       op0=mybir.AluOpType.mult,
                                       op1=mybir.AluOpType.add)
        sk = spool.tile([128, D], _F32, tag="sk")
        nc.vector.tensor_add(sk[:], paccs[:, 64:128], kg[:])

        # transposes + score matmul
        ptr = ppool.tile([128, 512], _F32, tag="ptr")
        nc.tensor.transpose(ptr[0:64, 256:384], sq[:], ident)
        sqT = spool.tile([64, 128], _F32, tag="sqT")
        nc.scalar.copy(sqT[:], ptr[0:64, 256:384])
        nc.tensor.transpose(ptr[0:64, 384:512], sk[:], ident)
        skpT = spool.tile([64, NJ + 3], _F32, tag="skpT")
        nc.vector.memset(skpT[:, 0:1], 0.0)
        nc.scalar.copy(skpT[:, 1:129], ptr[0:64, 384:512])

        nc.tensor.matmul(ptr[:, 0:NJ], sqT[:], skpT[:, 0:NJ], start=True, stop=True)

        # masked softmax
        Rm = spool.tile([128, NJ], _F32, tag="Rm")
        nc.vector.tensor_add(Rm[:], ptr[:, 0:NJ], maskneg)
        nm = spool.tile([128, 1], _F32, tag="nm")
        nc.vector.reduce_max(nm[:], Rm[:], axis=mybir.AxisListType.X, negate=True)
        e = spool.tile([128, NJ], _F32, tag="e")
        den = spool.tile([128, 1], _F32, tag="den")
        nc.scalar.activation(e[:], Rm[:], mybir.ActivationFunctionType.Exp,
                             bias=nm[:], scale=1.0, accum_out=den[:])
        rden = spool.tile([128, 1], _F32, tag="rden")
        nc.vector.reciprocal(rden[:], den[:])
        outb = spool.tile([128, NJ], _F32, tag="outb")
        nc.vector.scalar_tensor_tensor(outb[:], e[:], rden[:], maskstrict,
                                       op0=mybir.AluOpType.mult,
                                       op1=mybir.AluOpType.mult)
        nc.sync.dma_start(out[bh], outb[:])

    # software pipeline: bh's kg-reduce and tail phases are created after
    # bh+1's loads/plain-reduces so the DVE FIFO is never head-blocked on
    # the GPSIMD multiply chain.
    pend = None
    for bh in range(BH):
        st = stage_load(bh)
        if pend is not None:
            stage_finish(bh - 1, pend)
        pend = st
    stage_finish(BH - 1, pend)


_CACHE = {}


def _get_program():
    if "nc" not in _CACHE:
        _CACHE["nc"] = _build_program()
        _CACHE["consts"] = _host_constants()
    return _CACHE["nc"], _CACHE["consts"]


def _get_runner():
    """Build the sharded PJRT callable once and cache it (mirrors
    bass2jax.run_bass_via_pjrt but reuses the jitted function across
    calls)."""
    if "runner" in _CACHE:
        return _CACHE["runner"]
    import jax
    from jax.sharding import Mesh, PartitionSpec
    from jax.experimental.shard_map import shard_map
    from concourse import bass2jax

    nc, consts = _get_program()
    bass2jax.install_neuronx_cc_hook()

    part_name = nc.partition_id_tensor.name if nc.partition_id_tensor else None
    in_names, out_names, out_avals, zero_outs = [], [], [], []
    for alloc in nc.m.functions[0].allocations:
        if not isinstance(alloc, mybir.MemoryLocationSet):
            continue
        name = alloc.memorylocations[0].name
        if alloc.kind == "ExternalInput":
            if name != part_name:
                in_names.append(name)
        elif alloc.kind == "ExternalOutput":
            out_names.append(name)
            shape = tuple(alloc.tensor_shape)
            dtype = mybir.dt.np(alloc.dtype)
            out_avals.append(jax.core.ShapedArray(shape, dtype))
            zero_outs.append(np.zeros(shape, dtype))
    n_params = len(in_names)
    all_names = in_names + out_names
    if part_name is not None:
        all_names = all_names + [part_name]
    donate = tuple(range(n_params, n_params + len(out_names)))

    def _body(*args):
        operands = list(args)
        if part_name is not None:
            operands.append(bass2jax.partition_id_tensor())
        outs = bass2jax._bass_exec_p.bind(
            *operands,
            out_avals=tuple(out_avals),
            in_names=tuple(all_names),
            out_names=tuple(out_names),
            lowering_input_output_aliases=(),
            sim_require_finite=True,
            sim_require_nnan=True,
            nc=nc,
        )
        return tuple(outs)

    devices = jax.devices()[:N_CORES]
    mesh = Mesh(np.asarray(devices), ("core",))
    specs = (PartitionSpec("core"),) * (n_params + len(out_names))
    sharded = jax.jit(
        shard_map(_body, mesh=mesh, in_specs=specs,
                  out_specs=(PartitionSpec("core"),) * len(out_names),
                  check_rep=False),
        donate_argnums=donate, keep_unused=True,
    )
    runner = dict(fn=sharded, in_names=in_names, out_names=out_names,
                  zero_outs=zero_outs, consts=consts, nc=nc)
    _CACHE["runner"] = runner
    return runner


def _concat_inputs(q, k, runner):
    """Per-core input dict -> globally concatenated arrays (axis 0)."""
    consts = runner["consts"]
    arrs = []
    for name in runner["in_names"]:
        if name == "q":
            arrs.append(q)
        elif name == "k":
            arrs.append(k)
        else:
            c = consts[name]
            arrs.append(np.concatenate([c] * N_CORES, axis=0))
    return arrs


def kernel(q, k):
    q = np.ascontiguousarray(np.asarray(q, dtype=np.float32))
    k = np.ascontiguousarray(np.asarray(k, dtype=np.float32))
    assert q.shape == (BH_TOTAL, SEQ, D) and k.shape == (BH_TOTAL, SEQ, D)

    runner = _get_runner()
    # bh-shard across 8 cores: core c gets bh slice [4c, 4c+4). The global
    # concat layout [32, ...] already matches (shard_map splits axis 0).
    concat_in = _concat_inputs(q, k, runner)
    concat_zeros = [np.zeros((N_CORES * z.shape[0], *z.shape[1:]), z.dtype)
                    for z in runner["zero_outs"]]
    out_arrs = runner["fn"](*concat_in, *concat_zeros)
    out = np.asarray(out_arrs[0])          # [8*4, 128, 129]
    return np.ascontiguousarray(out.reshape(BH_TOTAL, NB, NJ))
